# revision 15
# baseline (speedup 1.0000x reference)
"""MoE layer (top-2 of 8 experts) on 8 Trainium2 NeuronCores.

Strategy: expert-parallel. The router (an [N,8] matmul), top-k dispatch and
final unshard/combine run on host as the shard/unshard steps; each core runs
the expert MLP  g * gelu(X_e @ W1_e + b1_e) @ W2_e  for the tokens routed to
its expert (capacity-padded), at 1/4 the dense-equivalent FLOPs.

Device dataflow (per core):
  - activations kept transposed (feature dim on partitions) so both matmuls
    chain without transposes:
      mm1: W1 tiles stationary, X^T moving  -> H^T (h on partitions)
      gelu(+b1) fused on ScalarE,  PSUM -> SBUF
      mm2: H^T subtiles stationary, W2 moving -> Y token-major
      gate scale fused into the PSUM->SBUF copy on ScalarE
  - matmuls in float32r (full-rate fp32 streaming mode)
  - all device inputs are host-pre-arranged into SBUF-native layouts so each
    DMA is one long contiguous run per partition (fast descriptor gen)
  - w1 streams in 8 chunks with the first two token tiles' mm1 interleaved
    across chunks, so the PE tracks the weight DMA at full duty from ~4us
Host then combines the two expert contributions per token (+b2, +residual)
and applies the LayerNorm; aux router losses are exact host-side math.
"""

import numpy as np

B, T, D, H, E, TOPK = 4, 2048, 1024, 2048, 8, 2
N = B * T
LN_EPS = 1e-5
P = 128
NTOK = 256  # tokens per mm1 moving tile
MM_DTYPE = "float32r"  # "float32r" (fast, ~1e-4 rel) or "float32" (exact)

KD = D // P  # 8   k-tiles over D
KH = H // P  # 16  k-tiles over H
NHC = 8  # w1 DMA chunks (2 h-tiles each)
HCH = KH // NHC  # h-tiles per chunk = 2
ND512 = D // 512  # 2

_CACHE = {}


def _build(C, b1_zero):
    import concourse.tile as tile
    from concourse import bacc, bass, mybir

    f32 = mybir.dt.float32
    fmm = getattr(mybir.dt, MM_DTYPE)
    NT = C // NTOK
    Gelu = mybir.ActivationFunctionType.Gelu
    Copy = mybir.ActivationFunctionType.Copy

    nc = bacc.Bacc("TRN2", target_bir_lowering=False, debug=False, num_devices=8)
    # all pre-arranged on host into SBUF-native layouts (partition dim = P)
    xt_d = nc.dram_tensor("xt", (NT, P, KD * NTOK), fmm, kind="ExternalInput").ap()
    w1_d = nc.dram_tensor("w1", (NHC, P, KD * HCH * P), fmm, kind="ExternalInput").ap()
    w2_d = nc.dram_tensor("w2", (ND512, P, KH * 512), fmm, kind="ExternalInput").ap()
    bg_d = nc.dram_tensor("bg", (P, KH + C // P), f32, kind="ExternalInput").ap()
    y_d = nc.dram_tensor("y", (C, D), f32, kind="ExternalOutput").ap()

    with tile.TileContext(nc) as tc:
        with (
            tc.tile_pool(name="wpool", bufs=1) as wpool,
            tc.tile_pool(name="xpool", bufs=3) as xpool,
            tc.tile_pool(name="hpool", bufs=3) as hpool,
            tc.tile_pool(name="ypool", bufs=1) as ypool,
            tc.tile_pool(name="ps1", bufs=3, space="PSUM") as ps1pool,
            tc.tile_pool(name="ps2", bufs=4, space="PSUM") as ps2pool,
        ):
            # weights stream on the sync engine; xts/y on the (idle) vector
            # engine so their issue+semaphore waves stay independent
            def dma_xts(tt):
                xts = xpool.tile([P, KD * NTOK], fmm, name="xts", tag="xts")
                nc.scalar.dma_start(xts[:], xt_d[tt])
                return xts

            # w1 chunk 0 first: the PE's first matmul needs only it + xts0
            w1cs = []

            def dma_w1c(hc):
                w1c = wpool.tile([P, KD * HCH * P], fmm, name=f"w1c{hc}")
                nc.sync.dma_start(w1c[:], w1_d[hc])
                w1cs.append(w1c)

            bgs = wpool.tile([P, KH + C // P], f32)
            nc.sync.dma_start(bgs[:], bg_d)
            b1s = bgs[:, :KH]
            gs = bgs[:, KH:]
            dma_w1c(0)
            NI = min(3, NT)
            xts_head = [dma_xts(t) for t in range(NI)]
            for hc in range(1, NHC):
                dma_w1c(hc)
            w2cs = []
            for dh in range(ND512):
                w2c = wpool.tile([P, KH * 512], fmm, name=f"w2c{dh}")
                nc.sync.dma_start(w2c[:], w2_d[dh])
                w2cs.append(w2c)

            hts_all = [None] * NT

            def mm1_hpair(hp, xts, hts):
                # two h-tiles accumulated into one PSUM bank, one gelu over
                # both (possible because b1 is zero -> shared scalar bias)
                ps1 = ps1pool.tile([P, 2 * NTOK], f32, name="ps1t", tag="ps1t")
                for half in range(2):
                    ht = 2 * hp + half
                    w1c = w1cs[ht // HCH]
                    hofs = (ht % HCH) * P
                    for kd in range(KD):
                        nc.tensor.matmul(
                            ps1[:, bass.ds(half * NTOK, NTOK)],
                            w1c[:, bass.ds(kd * HCH * P + hofs, P)],
                            xts[:, bass.ds(kd * NTOK, NTOK)],
                            start=(kd == 0),
                            stop=(kd == KD - 1),
                        )
                nc.scalar.activation(
                    hts[:, 2 * hp : 2 * hp + 2, :].rearrange("p a b -> p (a b)"),
                    ps1[:],
                    Gelu,
                )

            def mm1_htile(ht, xts, hts):
                ps1 = ps1pool.tile([P, NTOK], f32, name="ps1s", tag="ps1s")
                w1c = w1cs[ht // HCH]
                hofs = (ht % HCH) * P
                for kd in range(KD):
                    nc.tensor.matmul(
                        ps1[:],
                        w1c[:, bass.ds(kd * HCH * P + hofs, P)],
                        xts[:, bass.ds(kd * NTOK, NTOK)],
                        start=(kd == 0),
                        stop=(kd == KD - 1),
                    )
                nc.scalar.activation(
                    hts[:, ht, :], ps1[:], Gelu, bias=b1s[:, ht : ht + 1]
                )

            def mm1_chunk(hc, xts, hts):
                if b1_zero:
                    mm1_hpair(hc, xts, hts)
                else:
                    for ht in range(hc * HCH, (hc + 1) * HCH):
                        mm1_htile(ht, xts, hts)

            def emit_mm1(tt, xts):
                hts = hpool.tile([P, KH, NTOK], fmm, name="hts", tag="hts")
                for hc in range(NHC):
                    mm1_chunk(hc, xts, hts)
                hts_all[tt] = hts

            def emit_mm2(tt):
                hts = hts_all[tt]
                for sub in range(NTOK // P):
                    tsub = tt * (NTOK // P) + sub
                    ys = ypool.tile([P, D], f32, name="yst", tag="yst")
                    for dh in range(ND512):
                        ps2 = ps2pool.tile([P, 512], f32, name="ps2t", tag="ps2t")
                        for kh in range(KH):
                            nc.tensor.matmul(
                                ps2[:],
                                hts[:, kh, bass.ts(sub, P)],
                                w2cs[dh][:, bass.ds(kh * 512, 512)],
                                start=(kh == 0),
                                stop=(kh == KH - 1),
                            )
                        nc.vector.tensor_scalar_mul(
                            ys[:, bass.ts(dh, 512)],
                            ps2[:],
                            gs[:, tsub : tsub + 1],
                        )
                    nc.scalar.dma_start(y_d[bass.ts(tsub, P), :], ys[:])
                hts_all[tt] = None

            # startup: interleave the first NI tiles' mm1 across w1 chunks so
            # the PE tracks the w1 DMA stream instead of waiting for all of it
            hts_head = [
                hpool.tile([P, KH, NTOK], fmm, name="hts", tag="hts")
                for _ in range(NI)
            ]
            for hc in range(NHC):
                for t in range(NI):
                    mm1_chunk(hc, xts_head[t], hts_head[t])
            for t in range(NI):
                hts_all[t] = hts_head[t]

            # steady-state software pipeline, NI tiles of mm1 ahead of mm2
            for tt in range(NI, NT):
                emit_mm2(tt - NI)
                xts = dma_xts(tt)
                emit_mm1(tt, xts)
            for tt in range(max(NT - NI, 0), NT):
                emit_mm2(tt)
    nc.compile()
    return nc


def _get_program(C, b1_zero):
    key = (C, b1_zero)
    if key not in _CACHE:
        _CACHE[key] = _build(C, b1_zero)
    return _CACHE[key]


def kernel(x, gate_W, gate_b, W1, b1, W2, b2, ln_gamma, ln_beta):
    from concourse import bass_utils

    x = np.asarray(x, dtype=np.float32)
    gate_W = np.asarray(gate_W, dtype=np.float32)
    gate_b = np.asarray(gate_b, dtype=np.float32)
    W1 = np.asarray(W1, dtype=np.float32)
    b1 = np.asarray(b1, dtype=np.float32)
    W2 = np.asarray(W2, dtype=np.float32)
    b2 = np.asarray(b2, dtype=np.float32)
    ln_gamma = np.asarray(ln_gamma, dtype=np.float32)
    ln_beta = np.asarray(ln_beta, dtype=np.float32)

    tokens = x.reshape(N, D)

    # ---- router (host; this is the dispatch/sharding step) ----
    logits = tokens @ gate_W + gate_b  # [N, E] f32
    ar = np.arange(N)
    idx1 = np.argmax(logits, axis=1)
    masked = logits.copy()
    masked[ar, idx1] = -np.inf
    idx2 = np.argmax(masked, axis=1)
    v1 = logits[ar, idx1]
    v2 = masked[ar, idx2]
    # softmax over the two top values (stable; v1 >= v2)
    e2 = np.exp((v2 - v1).astype(np.float32))
    g1 = (1.0 / (1.0 + e2)).astype(np.float32)
    g2 = (e2 / (1.0 + e2)).astype(np.float32)

    # ---- per-expert dispatch (capacity-padded so shapes are static) ----
    sel1 = [np.flatnonzero(idx1 == e) for e in range(E)]
    sel2 = [np.flatnonzero(idx2 == e) for e in range(E)]
    counts = np.array([len(a) + len(b) for a, b in zip(sel1, sel2)])
    C = max(int(np.max(counts)), 1)
    C = ((C + NTOK - 1) // NTOK) * NTOK
    NT = C // NTOK

    pos1 = np.empty(N, dtype=np.int64)
    pos2 = np.empty(N, dtype=np.int64)
    in_maps = []
    for e in range(E):
        a, bsel = sel1[e], sel2[e]
        idx_e = np.concatenate([a, bsel])
        g_e = np.zeros(C, dtype=np.float32)
        g_e[: len(a)] = g1[a]
        g_e[len(a) : len(idx_e)] = g2[bsel]
        pos1[a] = e * C + np.arange(len(a))
        pos2[bsel] = e * C + len(a) + np.arange(len(bsel))
        xg = np.zeros((C, D), dtype=np.float32)
        xg[: len(idx_e)] = tokens[idx_e]
        # [NT, P, KD*NTOK]: xt[tt, p, kd*NTOK+c] = xg[tt*NTOK+c, kd*P+p]
        xt = np.ascontiguousarray(
            xg.reshape(NT, NTOK, KD, P).transpose(0, 3, 2, 1)
        ).reshape(NT, P, KD * NTOK)
        # [NHC, P, KD*HCH*P]: w1[hc, p, kd*HCH*P+hh] = W1[e][kd*P+p, hc*HCH*P+hh]
        w1 = np.ascontiguousarray(
            W1[e].reshape(KD, P, NHC, HCH * P).transpose(2, 1, 0, 3)
        ).reshape(NHC, P, KD * HCH * P)
        # [ND512, P, KH*512]: w2[dh, p, kh*512+dd] = W2[e][kh*P+p, dh*512+dd]
        w2 = np.ascontiguousarray(
            W2[e].reshape(KH, P, ND512, 512).transpose(2, 1, 0, 3)
        ).reshape(ND512, P, KH * 512)
        in_maps.append(
            {
                "xt": xt,
                "w1": w1,
                "w2": w2,
                "bg": np.ascontiguousarray(
                    np.concatenate(
                        [b1[e].reshape(KH, P).T, g_e.reshape(C // P, P).T], axis=1
                    )
                ),
            }
        )

    # ---- run the expert MLPs on the 8 cores ----
    nc = _get_program(C, not b1.any())
    res = bass_utils.run_bass_kernel_spmd(nc, in_maps, core_ids=list(range(E)))
    ycat = np.concatenate([r["y"] for r in res.results], axis=0)  # [E*C, D]

    # ---- unshard: combine the two expert contributions per token ----
    comb = ycat[pos1] + ycat[pos2]  # gates already applied on device
    z = comb + tokens
    if b2.any():
        z += b2[idx1] * g1[:, None] + b2[idx2] * g2[:, None]

    # ---- residual + layernorm (host) ----
    mu = z.mean(axis=1, keepdims=True, dtype=np.float32)
    zc = z - mu
    var = np.mean(zc * zc, axis=1, keepdims=True, dtype=np.float32)
    out = zc * (1.0 / np.sqrt(var + LN_EPS)) * ln_gamma + ln_beta
    out = out.reshape(B, T, D).astype(np.float32)

    # ---- aux losses (host, exact) ----
    m = logits.max(axis=1, keepdims=True)
    ee = np.exp(logits - m)
    route_probs = ee / ee.sum(axis=1, keepdims=True)  # [N, E] f32
    importance = route_probs.mean(axis=0).astype(np.float32)  # [E]
    load = (counts / N).astype(np.float32)  # exact: N = 2**13
    balance_loss = np.float32(E * np.sum(importance * load))
    eps = np.float32(1e-8)
    entropy = np.float32(-(route_probs * np.log(route_probs + eps)).sum(axis=-1).mean())
    utilization_entropy = np.float32(-(load * np.log(load + eps)).sum())

    return out, balance_loss, entropy, utilization_entropy, load, importance


# revision 16
# speedup vs baseline: 1.1845x; 1.1845x over previous
"""MoE layer (top-2 of 8 experts) on 8 Trainium2 NeuronCores.

Strategy: expert-parallel. The router (an [N,8] matmul), top-k dispatch and
final unshard/combine run on host as the shard/unshard steps; each core runs
the expert MLP  g * gelu(X_e @ W1_e + b1_e) @ W2_e  for the tokens routed to
its expert (capacity-padded), at 1/4 the dense-equivalent FLOPs.

Device dataflow (per core):
  - activations kept transposed (feature dim on partitions) so both matmuls
    chain without transposes:
      mm1: W1 tiles stationary, X^T moving  -> H^T (h on partitions)
      gelu(+b1) fused on ScalarE,  PSUM -> SBUF
      mm2: H^T subtiles stationary, W2 moving -> Y token-major
      gate scale fused into the PSUM->SBUF copy on ScalarE
  - matmuls in float32r (full-rate fp32 streaming mode)
  - all device inputs are host-pre-arranged into SBUF-native layouts so each
    DMA is one long contiguous run per partition (fast descriptor gen)
  - w1 streams in 8 chunks with the first two token tiles' mm1 interleaved
    across chunks, so the PE tracks the weight DMA at full duty from ~4us
Host then combines the two expert contributions per token (+b2, +residual)
and applies the LayerNorm; aux router losses are exact host-side math.
"""

import numpy as np

B, T, D, H, E, TOPK = 4, 2048, 1024, 2048, 8, 2
N = B * T
LN_EPS = 1e-5
P = 128
NTOK = 256  # tokens per mm1 moving tile
MM_DTYPE = "float32r"  # "float32r" (fast, ~1e-4 rel) or "float32" (exact)

KD = D // P  # 8   k-tiles over D
KH = H // P  # 16  k-tiles over H
NHC = 8  # w1 DMA chunks (2 h-tiles each)
HCH = KH // NHC  # h-tiles per chunk = 2
ND512 = D // 512  # 2

_CACHE = {}


def _build(C, b1_zero):
    import concourse.tile as tile
    from concourse import bacc, bass, mybir

    f32 = mybir.dt.float32
    fmm = getattr(mybir.dt, MM_DTYPE)
    NT = C // NTOK
    Gelu = mybir.ActivationFunctionType.Gelu
    Copy = mybir.ActivationFunctionType.Copy

    nc = bacc.Bacc("TRN2", target_bir_lowering=False, debug=False, num_devices=8)
    # all pre-arranged on host into SBUF-native layouts (partition dim = P)
    xt_d = nc.dram_tensor("xt", (NT, P, KD * NTOK), fmm, kind="ExternalInput").ap()
    w1_d = nc.dram_tensor("w1", (NHC, P, KD * HCH * P), fmm, kind="ExternalInput").ap()
    w2_d = nc.dram_tensor("w2", (ND512, P, KH * 512), fmm, kind="ExternalInput").ap()
    bg_d = nc.dram_tensor("bg", (P, KH + C // P), f32, kind="ExternalInput").ap()
    y_d = nc.dram_tensor("y", (C, D), f32, kind="ExternalOutput").ap()

    with tile.TileContext(nc) as tc:
        with (
            tc.tile_pool(name="wpool", bufs=1) as wpool,
            tc.tile_pool(name="xpool", bufs=3) as xpool,
            tc.tile_pool(name="hpool", bufs=3) as hpool,
            tc.tile_pool(name="ypool", bufs=1) as ypool,
            tc.tile_pool(name="ps1", bufs=4, space="PSUM") as ps1pool,
            tc.tile_pool(name="ps2", bufs=4, space="PSUM") as ps2pool,
        ):
            # weights stream on the sync engine; xts/y on the (idle) vector
            # engine so their issue+semaphore waves stay independent
            def dma_xts(tt):
                xts = xpool.tile([P, KD * NTOK], fmm, name="xts", tag="xts")
                nc.scalar.dma_start(xts[:], xt_d[tt])
                return xts

            # w1 chunk 0 first: the PE's first matmul needs only it + xts0
            w1cs = []

            def dma_w1c(hc):
                w1c = wpool.tile([P, KD * HCH * P], fmm, name=f"w1c{hc}")
                nc.sync.dma_start(w1c[:], w1_d[hc])
                w1cs.append(w1c)

            bgs = wpool.tile([P, KH + C // P], f32)
            nc.sync.dma_start(bgs[:], bg_d)
            b1s = bgs[:, :KH]
            gs = bgs[:, KH:]
            dma_w1c(0)
            NI = min(3, NT)
            xts_head = [dma_xts(t) for t in range(NI)]
            for hc in range(1, NHC):
                dma_w1c(hc)
            w2cs = []
            for dh in range(ND512):
                w2c = wpool.tile([P, KH * 512], fmm, name=f"w2c{dh}")
                nc.sync.dma_start(w2c[:], w2_d[dh])
                w2cs.append(w2c)

            hts_all = [None] * NT

            def mm1_hpair(hp, xts, hts):
                # two h-tiles accumulated into one PSUM bank, one gelu over
                # both (possible because b1 is zero -> shared scalar bias)
                ps1 = ps1pool.tile([P, 2 * NTOK], f32, name="ps1t", tag="ps1t")
                for half in range(2):
                    ht = 2 * hp + half
                    w1c = w1cs[ht // HCH]
                    hofs = (ht % HCH) * P
                    for kd in range(KD):
                        nc.tensor.matmul(
                            ps1[:, bass.ds(half * NTOK, NTOK)],
                            w1c[:, bass.ds(kd * HCH * P + hofs, P)],
                            xts[:, bass.ds(kd * NTOK, NTOK)],
                            start=(kd == 0),
                            stop=(kd == KD - 1),
                        )
                nc.scalar.activation(
                    hts[:, 2 * hp : 2 * hp + 2, :].rearrange("p a b -> p (a b)"),
                    ps1[:],
                    Gelu,
                )

            def mm1_htile(ht, xts, hts):
                ps1 = ps1pool.tile([P, NTOK], f32, name="ps1s", tag="ps1s")
                w1c = w1cs[ht // HCH]
                hofs = (ht % HCH) * P
                for kd in range(KD):
                    nc.tensor.matmul(
                        ps1[:],
                        w1c[:, bass.ds(kd * HCH * P + hofs, P)],
                        xts[:, bass.ds(kd * NTOK, NTOK)],
                        start=(kd == 0),
                        stop=(kd == KD - 1),
                    )
                nc.scalar.activation(
                    hts[:, ht, :], ps1[:], Gelu, bias=b1s[:, ht : ht + 1]
                )

            def mm1_chunk(hc, xts, hts):
                for ht in range(hc * HCH, (hc + 1) * HCH):
                    mm1_htile(ht, xts, hts)

            def emit_mm1(tt, xts):
                hts = hpool.tile([P, KH, NTOK], fmm, name="hts", tag="hts")
                for hc in range(NHC):
                    mm1_chunk(hc, xts, hts)
                hts_all[tt] = hts

            def emit_mm2(tt):
                hts = hts_all[tt]
                for sub in range(NTOK // P):
                    tsub = tt * (NTOK // P) + sub
                    ys = ypool.tile([P, D], f32, name="yst", tag="yst")
                    for dh in range(ND512):
                        ps2 = ps2pool.tile([P, 512], f32, name="ps2t", tag="ps2t")
                        for kh in range(KH):
                            nc.tensor.matmul(
                                ps2[:],
                                hts[:, kh, bass.ts(sub, P)],
                                w2cs[dh][:, bass.ds(kh * 512, 512)],
                                start=(kh == 0),
                                stop=(kh == KH - 1),
                            )
                        nc.vector.tensor_scalar_mul(
                            ys[:, bass.ts(dh, 512)],
                            ps2[:],
                            gs[:, tsub : tsub + 1],
                        )
                    nc.scalar.dma_start(y_d[bass.ts(tsub, P), :], ys[:])
                hts_all[tt] = None

            # startup: interleave the first NI tiles' mm1 across w1 chunks so
            # the PE tracks the w1 DMA stream instead of waiting for all of it
            hts_head = [
                hpool.tile([P, KH, NTOK], fmm, name="hts", tag="hts")
                for _ in range(NI)
            ]
            for hc in range(NHC):
                for t in range(NI):
                    mm1_chunk(hc, xts_head[t], hts_head[t])
            for t in range(NI):
                hts_all[t] = hts_head[t]

            # steady-state software pipeline, NI tiles of mm1 ahead of mm2
            for tt in range(NI, NT):
                emit_mm2(tt - NI)
                xts = dma_xts(tt)
                emit_mm1(tt, xts)
            for tt in range(max(NT - NI, 0), NT):
                emit_mm2(tt)
    nc.compile()
    return nc


def _get_program(C, b1_zero):
    key = (C, b1_zero)
    if key not in _CACHE:
        _CACHE[key] = _build(C, b1_zero)
    return _CACHE[key]


def kernel(x, gate_W, gate_b, W1, b1, W2, b2, ln_gamma, ln_beta):
    from concourse import bass_utils

    x = np.asarray(x, dtype=np.float32)
    gate_W = np.asarray(gate_W, dtype=np.float32)
    gate_b = np.asarray(gate_b, dtype=np.float32)
    W1 = np.asarray(W1, dtype=np.float32)
    b1 = np.asarray(b1, dtype=np.float32)
    W2 = np.asarray(W2, dtype=np.float32)
    b2 = np.asarray(b2, dtype=np.float32)
    ln_gamma = np.asarray(ln_gamma, dtype=np.float32)
    ln_beta = np.asarray(ln_beta, dtype=np.float32)

    tokens = x.reshape(N, D)

    # ---- router (host; this is the dispatch/sharding step) ----
    logits = tokens @ gate_W + gate_b  # [N, E] f32
    ar = np.arange(N)
    idx1 = np.argmax(logits, axis=1)
    masked = logits.copy()
    masked[ar, idx1] = -np.inf
    idx2 = np.argmax(masked, axis=1)
    v1 = logits[ar, idx1]
    v2 = masked[ar, idx2]
    # softmax over the two top values (stable; v1 >= v2)
    e2 = np.exp((v2 - v1).astype(np.float32))
    g1 = (1.0 / (1.0 + e2)).astype(np.float32)
    g2 = (e2 / (1.0 + e2)).astype(np.float32)

    # ---- per-expert dispatch (capacity-padded so shapes are static) ----
    sel1 = [np.flatnonzero(idx1 == e) for e in range(E)]
    sel2 = [np.flatnonzero(idx2 == e) for e in range(E)]
    counts = np.array([len(a) + len(b) for a, b in zip(sel1, sel2)])
    C = max(int(np.max(counts)), 1)
    C = ((C + NTOK - 1) // NTOK) * NTOK
    NT = C // NTOK

    pos1 = np.empty(N, dtype=np.int64)
    pos2 = np.empty(N, dtype=np.int64)
    in_maps = []
    for e in range(E):
        a, bsel = sel1[e], sel2[e]
        idx_e = np.concatenate([a, bsel])
        g_e = np.zeros(C, dtype=np.float32)
        g_e[: len(a)] = g1[a]
        g_e[len(a) : len(idx_e)] = g2[bsel]
        pos1[a] = e * C + np.arange(len(a))
        pos2[bsel] = e * C + len(a) + np.arange(len(bsel))
        xg = np.zeros((C, D), dtype=np.float32)
        xg[: len(idx_e)] = tokens[idx_e]
        # [NT, P, KD*NTOK]: xt[tt, p, kd*NTOK+c] = xg[tt*NTOK+c, kd*P+p]
        xt = np.ascontiguousarray(
            xg.reshape(NT, NTOK, KD, P).transpose(0, 3, 2, 1)
        ).reshape(NT, P, KD * NTOK)
        # [NHC, P, KD*HCH*P]: w1[hc, p, kd*HCH*P+hh] = W1[e][kd*P+p, hc*HCH*P+hh]
        w1 = np.ascontiguousarray(
            W1[e].reshape(KD, P, NHC, HCH * P).transpose(2, 1, 0, 3)
        ).reshape(NHC, P, KD * HCH * P)
        # [ND512, P, KH*512]: w2[dh, p, kh*512+dd] = W2[e][kh*P+p, dh*512+dd]
        w2 = np.ascontiguousarray(
            W2[e].reshape(KH, P, ND512, 512).transpose(2, 1, 0, 3)
        ).reshape(ND512, P, KH * 512)
        in_maps.append(
            {
                "xt": xt,
                "w1": w1,
                "w2": w2,
                "bg": np.ascontiguousarray(
                    np.concatenate(
                        [b1[e].reshape(KH, P).T, g_e.reshape(C // P, P).T], axis=1
                    )
                ),
            }
        )

    # ---- run the expert MLPs on the 8 cores ----
    nc = _get_program(C, not b1.any())
    res = bass_utils.run_bass_kernel_spmd(nc, in_maps, core_ids=list(range(E)))
    ycat = np.concatenate([r["y"] for r in res.results], axis=0)  # [E*C, D]

    # ---- unshard: combine the two expert contributions per token ----
    comb = ycat[pos1] + ycat[pos2]  # gates already applied on device
    z = comb + tokens
    if b2.any():
        z += b2[idx1] * g1[:, None] + b2[idx2] * g2[:, None]

    # ---- residual + layernorm (host) ----
    mu = z.mean(axis=1, keepdims=True, dtype=np.float32)
    zc = z - mu
    var = np.mean(zc * zc, axis=1, keepdims=True, dtype=np.float32)
    out = zc * (1.0 / np.sqrt(var + LN_EPS)) * ln_gamma + ln_beta
    out = out.reshape(B, T, D).astype(np.float32)

    # ---- aux losses (host, exact) ----
    m = logits.max(axis=1, keepdims=True)
    ee = np.exp(logits - m)
    route_probs = ee / ee.sum(axis=1, keepdims=True)  # [N, E] f32
    importance = route_probs.mean(axis=0).astype(np.float32)  # [E]
    load = (counts / N).astype(np.float32)  # exact: N = 2**13
    balance_loss = np.float32(E * np.sum(importance * load))
    eps = np.float32(1e-8)
    entropy = np.float32(-(route_probs * np.log(route_probs + eps)).sum(axis=-1).mean())
    utilization_entropy = np.float32(-(load * np.log(load + eps)).sum())

    return out, balance_loss, entropy, utilization_entropy, load, importance


# revision 17
# speedup vs baseline: 1.2086x; 1.0203x over previous
"""MoE layer (top-2 of 8 experts) on 8 Trainium2 NeuronCores.

Strategy: expert-parallel. The router (an [N,8] matmul), top-k dispatch and
final unshard/combine run on host as the shard/unshard steps; each core runs
the expert MLP  g * gelu(X_e @ W1_e + b1_e) @ W2_e  for the tokens routed to
its expert (capacity-padded), at 1/4 the dense-equivalent FLOPs.

Device dataflow (per core):
  - activations kept transposed (feature dim on partitions) so both matmuls
    chain without transposes:
      mm1: W1 tiles stationary, X^T moving  -> H^T (h on partitions)
      gelu(+b1) fused on ScalarE,  PSUM -> SBUF
      mm2: H^T subtiles stationary, W2 moving -> Y token-major
      gate scale fused into the PSUM->SBUF copy on ScalarE
  - matmuls in float32r (full-rate fp32 streaming mode)
  - all device inputs are host-pre-arranged into SBUF-native layouts so each
    DMA is one long contiguous run per partition (fast descriptor gen)
  - w1 streams in 8 chunks with the first two token tiles' mm1 interleaved
    across chunks, so the PE tracks the weight DMA at full duty from ~4us
Host then combines the two expert contributions per token (+b2, +residual)
and applies the LayerNorm; aux router losses are exact host-side math.
"""

import numpy as np

B, T, D, H, E, TOPK = 4, 2048, 1024, 2048, 8, 2
N = B * T
LN_EPS = 1e-5
P = 128
NTOK = 384  # tokens per mm1 moving tile
MM_DTYPE = "float32r"  # "float32r" (fast, ~1e-4 rel) or "float32" (exact)

KD = D // P  # 8   k-tiles over D
KH = H // P  # 16  k-tiles over H
NHC = 16  # w1 DMA chunks (1 h-tile each)
HCH = KH // NHC  # h-tiles per chunk = 1
ND512 = D // 512  # 2

_CACHE = {}


def _build(C, b1_zero):
    import concourse.tile as tile
    from concourse import bacc, bass, mybir

    f32 = mybir.dt.float32
    fmm = getattr(mybir.dt, MM_DTYPE)
    NT = C // NTOK
    Gelu = mybir.ActivationFunctionType.Gelu
    Copy = mybir.ActivationFunctionType.Copy

    nc = bacc.Bacc("TRN2", target_bir_lowering=False, debug=False, num_devices=8)
    # all pre-arranged on host into SBUF-native layouts (partition dim = P)
    xt_d = nc.dram_tensor("xt", (NT, P, KD * NTOK), fmm, kind="ExternalInput").ap()
    w1_d = nc.dram_tensor("w1", (NHC, P, KD * HCH * P), fmm, kind="ExternalInput").ap()
    w2_d = nc.dram_tensor("w2", (ND512, P, KH * 512), fmm, kind="ExternalInput").ap()
    bg_d = nc.dram_tensor("bg", (P, KH + C // P), f32, kind="ExternalInput").ap()
    y_d = nc.dram_tensor("y", (C, D), f32, kind="ExternalOutput").ap()

    with tile.TileContext(nc) as tc:
        with (
            tc.tile_pool(name="wpool", bufs=1) as wpool,
            tc.tile_pool(name="xpool", bufs=2) as xpool,
            tc.tile_pool(name="hpool", bufs=2) as hpool,
            tc.tile_pool(name="ypool", bufs=1) as ypool,
            tc.tile_pool(name="ps1", bufs=4, space="PSUM") as ps1pool,
            tc.tile_pool(name="ps2", bufs=4, space="PSUM") as ps2pool,
        ):
            # weights stream on the sync engine; xts/y on the (idle) vector
            # engine so their issue+semaphore waves stay independent
            def dma_xts(tt):
                xts = xpool.tile([P, KD * NTOK], fmm, name="xts", tag="xts")
                nc.scalar.dma_start(xts[:], xt_d[tt])
                return xts

            # w1 chunk 0 first: the PE's first matmul needs only it + xts0
            w1cs = []

            def dma_w1c(hc):
                w1c = wpool.tile([P, KD * HCH * P], fmm, name=f"w1c{hc}")
                nc.sync.dma_start(w1c[:], w1_d[hc])
                w1cs.append(w1c)

            bgs = wpool.tile([P, KH + C // P], f32)
            nc.sync.dma_start(bgs[:], bg_d)
            b1s = bgs[:, :KH]
            gs = bgs[:, KH:]
            dma_w1c(0)
            NI = min(2, NT)
            xts_head = [dma_xts(t) for t in range(NI)]
            for hc in range(1, NHC):
                dma_w1c(hc)
            w2cs = []
            for dh in range(ND512):
                w2c = wpool.tile([P, KH * 512], fmm, name=f"w2c{dh}")
                nc.sync.dma_start(w2c[:], w2_d[dh])
                w2cs.append(w2c)

            hts_all = [None] * NT

            def mm1_hpair(hp, xts, hts):
                # two h-tiles accumulated into one PSUM bank, one gelu over
                # both (possible because b1 is zero -> shared scalar bias)
                ps1 = ps1pool.tile([P, 2 * NTOK], f32, name="ps1t", tag="ps1t")
                for half in range(2):
                    ht = 2 * hp + half
                    w1c = w1cs[ht // HCH]
                    hofs = (ht % HCH) * P
                    for kd in range(KD):
                        nc.tensor.matmul(
                            ps1[:, bass.ds(half * NTOK, NTOK)],
                            w1c[:, bass.ds(kd * HCH * P + hofs, P)],
                            xts[:, bass.ds(kd * NTOK, NTOK)],
                            start=(kd == 0),
                            stop=(kd == KD - 1),
                        )
                nc.scalar.activation(
                    hts[:, 2 * hp : 2 * hp + 2, :].rearrange("p a b -> p (a b)"),
                    ps1[:],
                    Gelu,
                )

            def mm1_htile(ht, xts, hts):
                ps1 = ps1pool.tile([P, NTOK], f32, name="ps1s", tag="ps1s")
                w1c = w1cs[ht // HCH]
                hofs = (ht % HCH) * P
                for kd in range(KD):
                    nc.tensor.matmul(
                        ps1[:],
                        w1c[:, bass.ds(kd * HCH * P + hofs, P)],
                        xts[:, bass.ds(kd * NTOK, NTOK)],
                        start=(kd == 0),
                        stop=(kd == KD - 1),
                    )
                nc.scalar.activation(
                    hts[:, ht, :], ps1[:], Gelu, bias=b1s[:, ht : ht + 1]
                )

            def mm1_chunk(hc, xts, hts):
                for ht in range(hc * HCH, (hc + 1) * HCH):
                    mm1_htile(ht, xts, hts)

            def emit_mm1(tt, xts):
                hts = hpool.tile([P, KH, NTOK], fmm, name="hts", tag="hts")
                for hc in range(NHC):
                    mm1_chunk(hc, xts, hts)
                hts_all[tt] = hts

            def emit_mm2(tt):
                hts = hts_all[tt]
                for sub in range(NTOK // P):
                    tsub = tt * (NTOK // P) + sub
                    ys = ypool.tile([P, D], f32, name="yst", tag="yst")
                    for dh in range(ND512):
                        ps2 = ps2pool.tile([P, 512], f32, name="ps2t", tag="ps2t")
                        for kh in range(KH):
                            nc.tensor.matmul(
                                ps2[:],
                                hts[:, kh, bass.ts(sub, P)],
                                w2cs[dh][:, bass.ds(kh * 512, 512)],
                                start=(kh == 0),
                                stop=(kh == KH - 1),
                            )
                        nc.vector.tensor_scalar_mul(
                            ys[:, bass.ts(dh, 512)],
                            ps2[:],
                            gs[:, tsub : tsub + 1],
                        )
                    nc.scalar.dma_start(y_d[bass.ts(tsub, P), :], ys[:])
                hts_all[tt] = None

            # startup: interleave the first NI tiles' mm1 across w1 chunks so
            # the PE tracks the w1 DMA stream instead of waiting for all of it
            hts_head = [
                hpool.tile([P, KH, NTOK], fmm, name="hts", tag="hts")
                for _ in range(NI)
            ]
            for hc in range(NHC):
                for t in range(NI):
                    mm1_chunk(hc, xts_head[t], hts_head[t])
            for t in range(NI):
                hts_all[t] = hts_head[t]

            # steady-state software pipeline, NI tiles of mm1 ahead of mm2
            for tt in range(NI, NT):
                emit_mm2(tt - NI)
                xts = dma_xts(tt)
                emit_mm1(tt, xts)
            for tt in range(max(NT - NI, 0), NT):
                emit_mm2(tt)
    nc.compile()
    return nc


def _get_program(C, b1_zero):
    key = (C, b1_zero)
    if key not in _CACHE:
        _CACHE[key] = _build(C, b1_zero)
    return _CACHE[key]


def kernel(x, gate_W, gate_b, W1, b1, W2, b2, ln_gamma, ln_beta):
    from concourse import bass_utils

    x = np.asarray(x, dtype=np.float32)
    gate_W = np.asarray(gate_W, dtype=np.float32)
    gate_b = np.asarray(gate_b, dtype=np.float32)
    W1 = np.asarray(W1, dtype=np.float32)
    b1 = np.asarray(b1, dtype=np.float32)
    W2 = np.asarray(W2, dtype=np.float32)
    b2 = np.asarray(b2, dtype=np.float32)
    ln_gamma = np.asarray(ln_gamma, dtype=np.float32)
    ln_beta = np.asarray(ln_beta, dtype=np.float32)

    tokens = x.reshape(N, D)

    # ---- router (host; this is the dispatch/sharding step) ----
    logits = tokens @ gate_W + gate_b  # [N, E] f32
    ar = np.arange(N)
    idx1 = np.argmax(logits, axis=1)
    masked = logits.copy()
    masked[ar, idx1] = -np.inf
    idx2 = np.argmax(masked, axis=1)
    v1 = logits[ar, idx1]
    v2 = masked[ar, idx2]
    # softmax over the two top values (stable; v1 >= v2)
    e2 = np.exp((v2 - v1).astype(np.float32))
    g1 = (1.0 / (1.0 + e2)).astype(np.float32)
    g2 = (e2 / (1.0 + e2)).astype(np.float32)

    # ---- per-expert dispatch (capacity-padded so shapes are static) ----
    sel1 = [np.flatnonzero(idx1 == e) for e in range(E)]
    sel2 = [np.flatnonzero(idx2 == e) for e in range(E)]
    counts = np.array([len(a) + len(b) for a, b in zip(sel1, sel2)])
    C = max(int(np.max(counts)), 1)
    C = ((C + NTOK - 1) // NTOK) * NTOK
    NT = C // NTOK

    pos1 = np.empty(N, dtype=np.int64)
    pos2 = np.empty(N, dtype=np.int64)
    in_maps = []
    for e in range(E):
        a, bsel = sel1[e], sel2[e]
        idx_e = np.concatenate([a, bsel])
        g_e = np.zeros(C, dtype=np.float32)
        g_e[: len(a)] = g1[a]
        g_e[len(a) : len(idx_e)] = g2[bsel]
        pos1[a] = e * C + np.arange(len(a))
        pos2[bsel] = e * C + len(a) + np.arange(len(bsel))
        xg = np.zeros((C, D), dtype=np.float32)
        xg[: len(idx_e)] = tokens[idx_e]
        # [NT, P, KD*NTOK]: xt[tt, p, kd*NTOK+c] = xg[tt*NTOK+c, kd*P+p]
        xt = np.ascontiguousarray(
            xg.reshape(NT, NTOK, KD, P).transpose(0, 3, 2, 1)
        ).reshape(NT, P, KD * NTOK)
        # [NHC, P, KD*HCH*P]: w1[hc, p, kd*HCH*P+hh] = W1[e][kd*P+p, hc*HCH*P+hh]
        w1 = np.ascontiguousarray(
            W1[e].reshape(KD, P, NHC, HCH * P).transpose(2, 1, 0, 3)
        ).reshape(NHC, P, KD * HCH * P)
        # [ND512, P, KH*512]: w2[dh, p, kh*512+dd] = W2[e][kh*P+p, dh*512+dd]
        w2 = np.ascontiguousarray(
            W2[e].reshape(KH, P, ND512, 512).transpose(2, 1, 0, 3)
        ).reshape(ND512, P, KH * 512)
        in_maps.append(
            {
                "xt": xt,
                "w1": w1,
                "w2": w2,
                "bg": np.ascontiguousarray(
                    np.concatenate(
                        [b1[e].reshape(KH, P).T, g_e.reshape(C // P, P).T], axis=1
                    )
                ),
            }
        )

    # ---- run the expert MLPs on the 8 cores ----
    nc = _get_program(C, not b1.any())
    res = bass_utils.run_bass_kernel_spmd(nc, in_maps, core_ids=list(range(E)))
    ycat = np.concatenate([r["y"] for r in res.results], axis=0)  # [E*C, D]

    # ---- unshard: combine the two expert contributions per token ----
    comb = ycat[pos1] + ycat[pos2]  # gates already applied on device
    z = comb + tokens
    if b2.any():
        z += b2[idx1] * g1[:, None] + b2[idx2] * g2[:, None]

    # ---- residual + layernorm (host) ----
    mu = z.mean(axis=1, keepdims=True, dtype=np.float32)
    zc = z - mu
    var = np.mean(zc * zc, axis=1, keepdims=True, dtype=np.float32)
    out = zc * (1.0 / np.sqrt(var + LN_EPS)) * ln_gamma + ln_beta
    out = out.reshape(B, T, D).astype(np.float32)

    # ---- aux losses (host, exact) ----
    m = logits.max(axis=1, keepdims=True)
    ee = np.exp(logits - m)
    route_probs = ee / ee.sum(axis=1, keepdims=True)  # [N, E] f32
    importance = route_probs.mean(axis=0).astype(np.float32)  # [E]
    load = (counts / N).astype(np.float32)  # exact: N = 2**13
    balance_loss = np.float32(E * np.sum(importance * load))
    eps = np.float32(1e-8)
    entropy = np.float32(-(route_probs * np.log(route_probs + eps)).sum(axis=-1).mean())
    utilization_entropy = np.float32(-(load * np.log(load + eps)).sum())

    return out, balance_loss, entropy, utilization_entropy, load, importance


# revision 19
# speedup vs baseline: 1.2142x; 1.0047x over previous
"""MoE layer (top-2 of 8 experts) on 8 Trainium2 NeuronCores.

Strategy: expert-parallel. The router (an [N,8] matmul), top-k dispatch and
final unshard/combine run on host as the shard/unshard steps; each core runs
the expert MLP  g * gelu(X_e @ W1_e + b1_e) @ W2_e  for the tokens routed to
its expert (capacity-padded), at 1/4 the dense-equivalent FLOPs.

Device dataflow (per core):
  - activations kept transposed (feature dim on partitions) so both matmuls
    chain without transposes:
      mm1: W1 tiles stationary, X^T moving  -> H^T (h on partitions)
      gelu(+b1) fused on ScalarE,  PSUM -> SBUF
      mm2: H^T subtiles stationary, W2 moving -> Y token-major
      gate scale fused into the PSUM->SBUF copy on ScalarE
  - matmuls in float32r (full-rate fp32 streaming mode)
  - all device inputs are host-pre-arranged into SBUF-native layouts so each
    DMA is one long contiguous run per partition (fast descriptor gen)
  - w1 streams in 8 chunks with the first two token tiles' mm1 interleaved
    across chunks, so the PE tracks the weight DMA at full duty from ~4us
Host then combines the two expert contributions per token (+b2, +residual)
and applies the LayerNorm; aux router losses are exact host-side math.
"""

import numpy as np

B, T, D, H, E, TOPK = 4, 2048, 1024, 2048, 8, 2
N = B * T
LN_EPS = 1e-5
P = 128
NTOK = 384  # tokens per mm1 moving tile
MM_DTYPE = "float32r"  # "float32r" (fast, ~1e-4 rel) or "float32" (exact)

KD = D // P  # 8   k-tiles over D
KH = H // P  # 16  k-tiles over H
NHC = 16  # w1 DMA chunks (1 h-tile each)
HCH = KH // NHC  # h-tiles per chunk = 1
ND512 = D // 512  # 2

_CACHE = {}


def _build(C, b1_zero):
    import concourse.tile as tile
    from concourse import bacc, bass, mybir

    f32 = mybir.dt.float32
    fmm = getattr(mybir.dt, MM_DTYPE)
    NT = C // NTOK
    Gelu = mybir.ActivationFunctionType.Gelu
    Copy = mybir.ActivationFunctionType.Copy

    nc = bacc.Bacc("TRN2", target_bir_lowering=False, debug=False, num_devices=8)
    # all pre-arranged on host into SBUF-native layouts (partition dim = P)
    xt_d = nc.dram_tensor("xt", (NT, P, KD * NTOK), fmm, kind="ExternalInput").ap()
    w1_d = nc.dram_tensor("w1", (NHC, P, KD * HCH * P), fmm, kind="ExternalInput").ap()
    w2_d = nc.dram_tensor("w2", (ND512, P, KH * 512), fmm, kind="ExternalInput").ap()
    bg_d = nc.dram_tensor("bg", (P, KH + C // P), f32, kind="ExternalInput").ap()
    y_d = nc.dram_tensor("y", (C, D), f32, kind="ExternalOutput").ap()

    with tile.TileContext(nc) as tc:
        with (
            tc.tile_pool(name="wpool", bufs=1) as wpool,
            tc.tile_pool(name="xpool", bufs=2) as xpool,
            tc.tile_pool(name="hpool", bufs=2) as hpool,
            tc.tile_pool(name="ypool", bufs=1) as ypool,
            tc.tile_pool(name="ps1", bufs=4, space="PSUM") as ps1pool,
            tc.tile_pool(name="ps2", bufs=4, space="PSUM") as ps2pool,
        ):
            # weights stream on the sync engine; xts/y on the (idle) vector
            # engine so their issue+semaphore waves stay independent
            def dma_xts(tt):
                xts = xpool.tile([P, KD * NTOK], fmm, name="xts", tag="xts")
                nc.scalar.dma_start(xts[:], xt_d[tt])
                return xts

            # w1 chunk 0 first: the PE's first matmul needs only it + xts0
            w1cs = []

            def dma_w1c(hc):
                w1c = wpool.tile([P, KD * HCH * P], fmm, name=f"w1c{hc}")
                nc.sync.dma_start(w1c[:], w1_d[hc])
                w1cs.append(w1c)

            bgs = wpool.tile([P, KH + C // P], f32)
            nc.sync.dma_start(bgs[:], bg_d)
            b1s = bgs[:, :KH]
            gs = bgs[:, KH:]
            dma_w1c(0)
            NI = min(2, NT)
            xts_head = [dma_xts(t) for t in range(NI)]
            for hc in range(1, NHC):
                dma_w1c(hc)
            w2cs = []
            for dh in range(ND512):
                w2c = wpool.tile([P, KH * 512], fmm, name=f"w2c{dh}")
                nc.sync.dma_start(w2c[:], w2_d[dh])
                w2cs.append(w2c)

            # dummy matmuls on zeroed SBUF: keep the PE busy (and the HAM
            # clock-gate warm) while the first real input DMAs stream in
            dw = wpool.tile([P, P], f32)
            nc.gpsimd.memset(dw[:], 0.0)
            dx = wpool.tile([P, NTOK], f32)
            nc.gpsimd.memset(dx[:], 0.0)
            for _g in range(2):
                dps = ps1pool.tile([P, NTOK], f32, name="ps1s", tag="ps1s")
                for k in range(5):
                    nc.tensor.matmul(
                        dps[:], dw[:], dx[:], start=(k == 0), stop=(k == 4)
                    )

            hts_all = [None] * NT

            def mm1_hpair(hp, xts, hts):
                # two h-tiles accumulated into one PSUM bank, one gelu over
                # both (possible because b1 is zero -> shared scalar bias)
                ps1 = ps1pool.tile([P, 2 * NTOK], f32, name="ps1t", tag="ps1t")
                for half in range(2):
                    ht = 2 * hp + half
                    w1c = w1cs[ht // HCH]
                    hofs = (ht % HCH) * P
                    for kd in range(KD):
                        nc.tensor.matmul(
                            ps1[:, bass.ds(half * NTOK, NTOK)],
                            w1c[:, bass.ds(kd * HCH * P + hofs, P)],
                            xts[:, bass.ds(kd * NTOK, NTOK)],
                            start=(kd == 0),
                            stop=(kd == KD - 1),
                        )
                nc.scalar.activation(
                    hts[:, 2 * hp : 2 * hp + 2, :].rearrange("p a b -> p (a b)"),
                    ps1[:],
                    Gelu,
                )

            def mm1_htile(ht, xts, hts):
                ps1 = ps1pool.tile([P, NTOK], f32, name="ps1s", tag="ps1s")
                w1c = w1cs[ht // HCH]
                hofs = (ht % HCH) * P
                for kd in range(KD):
                    nc.tensor.matmul(
                        ps1[:],
                        w1c[:, bass.ds(kd * HCH * P + hofs, P)],
                        xts[:, bass.ds(kd * NTOK, NTOK)],
                        start=(kd == 0),
                        stop=(kd == KD - 1),
                    )
                nc.scalar.activation(
                    hts[:, ht, :], ps1[:], Gelu, bias=b1s[:, ht : ht + 1]
                )

            def mm1_chunk(hc, xts, hts):
                for ht in range(hc * HCH, (hc + 1) * HCH):
                    mm1_htile(ht, xts, hts)

            def emit_mm1(tt, xts):
                hts = hpool.tile([P, KH, NTOK], fmm, name="hts", tag="hts")
                for hc in range(NHC):
                    mm1_chunk(hc, xts, hts)
                hts_all[tt] = hts

            def emit_mm2(tt):
                hts = hts_all[tt]
                for sub in range(NTOK // P):
                    tsub = tt * (NTOK // P) + sub
                    ys = ypool.tile([P, D], f32, name="yst", tag="yst")
                    for dh in range(ND512):
                        ps2 = ps2pool.tile([P, 512], f32, name="ps2t", tag="ps2t")
                        for kh in range(KH):
                            nc.tensor.matmul(
                                ps2[:],
                                hts[:, kh, bass.ts(sub, P)],
                                w2cs[dh][:, bass.ds(kh * 512, 512)],
                                start=(kh == 0),
                                stop=(kh == KH - 1),
                            )
                        nc.vector.tensor_scalar_mul(
                            ys[:, bass.ts(dh, 512)],
                            ps2[:],
                            gs[:, tsub : tsub + 1],
                        )
                    nc.scalar.dma_start(y_d[bass.ts(tsub, P), :], ys[:])
                hts_all[tt] = None

            # startup: interleave the first NI tiles' mm1 across w1 chunks so
            # the PE tracks the w1 DMA stream instead of waiting for all of it
            hts_head = [
                hpool.tile([P, KH, NTOK], fmm, name="hts", tag="hts")
                for _ in range(NI)
            ]
            for hc in range(NHC):
                for t in range(NI):
                    mm1_chunk(hc, xts_head[t], hts_head[t])
            for t in range(NI):
                hts_all[t] = hts_head[t]

            # steady-state software pipeline, NI tiles of mm1 ahead of mm2
            for tt in range(NI, NT):
                emit_mm2(tt - NI)
                xts = dma_xts(tt)
                emit_mm1(tt, xts)
            for tt in range(max(NT - NI, 0), NT):
                emit_mm2(tt)
    nc.compile()
    return nc


def _get_program(C, b1_zero):
    key = (C, b1_zero)
    if key not in _CACHE:
        _CACHE[key] = _build(C, b1_zero)
    return _CACHE[key]


def kernel(x, gate_W, gate_b, W1, b1, W2, b2, ln_gamma, ln_beta):
    from concourse import bass_utils

    x = np.asarray(x, dtype=np.float32)
    gate_W = np.asarray(gate_W, dtype=np.float32)
    gate_b = np.asarray(gate_b, dtype=np.float32)
    W1 = np.asarray(W1, dtype=np.float32)
    b1 = np.asarray(b1, dtype=np.float32)
    W2 = np.asarray(W2, dtype=np.float32)
    b2 = np.asarray(b2, dtype=np.float32)
    ln_gamma = np.asarray(ln_gamma, dtype=np.float32)
    ln_beta = np.asarray(ln_beta, dtype=np.float32)

    tokens = x.reshape(N, D)

    # ---- router (host; this is the dispatch/sharding step) ----
    logits = tokens @ gate_W + gate_b  # [N, E] f32
    ar = np.arange(N)
    idx1 = np.argmax(logits, axis=1)
    masked = logits.copy()
    masked[ar, idx1] = -np.inf
    idx2 = np.argmax(masked, axis=1)
    v1 = logits[ar, idx1]
    v2 = masked[ar, idx2]
    # softmax over the two top values (stable; v1 >= v2)
    e2 = np.exp((v2 - v1).astype(np.float32))
    g1 = (1.0 / (1.0 + e2)).astype(np.float32)
    g2 = (e2 / (1.0 + e2)).astype(np.float32)

    # ---- per-expert dispatch (capacity-padded so shapes are static) ----
    sel1 = [np.flatnonzero(idx1 == e) for e in range(E)]
    sel2 = [np.flatnonzero(idx2 == e) for e in range(E)]
    counts = np.array([len(a) + len(b) for a, b in zip(sel1, sel2)])
    C = max(int(np.max(counts)), 1)
    C = ((C + NTOK - 1) // NTOK) * NTOK
    NT = C // NTOK

    pos1 = np.empty(N, dtype=np.int64)
    pos2 = np.empty(N, dtype=np.int64)
    in_maps = []
    for e in range(E):
        a, bsel = sel1[e], sel2[e]
        idx_e = np.concatenate([a, bsel])
        g_e = np.zeros(C, dtype=np.float32)
        g_e[: len(a)] = g1[a]
        g_e[len(a) : len(idx_e)] = g2[bsel]
        pos1[a] = e * C + np.arange(len(a))
        pos2[bsel] = e * C + len(a) + np.arange(len(bsel))
        xg = np.zeros((C, D), dtype=np.float32)
        xg[: len(idx_e)] = tokens[idx_e]
        # [NT, P, KD*NTOK]: xt[tt, p, kd*NTOK+c] = xg[tt*NTOK+c, kd*P+p]
        xt = np.ascontiguousarray(
            xg.reshape(NT, NTOK, KD, P).transpose(0, 3, 2, 1)
        ).reshape(NT, P, KD * NTOK)
        # [NHC, P, KD*HCH*P]: w1[hc, p, kd*HCH*P+hh] = W1[e][kd*P+p, hc*HCH*P+hh]
        w1 = np.ascontiguousarray(
            W1[e].reshape(KD, P, NHC, HCH * P).transpose(2, 1, 0, 3)
        ).reshape(NHC, P, KD * HCH * P)
        # [ND512, P, KH*512]: w2[dh, p, kh*512+dd] = W2[e][kh*P+p, dh*512+dd]
        w2 = np.ascontiguousarray(
            W2[e].reshape(KH, P, ND512, 512).transpose(2, 1, 0, 3)
        ).reshape(ND512, P, KH * 512)
        in_maps.append(
            {
                "xt": xt,
                "w1": w1,
                "w2": w2,
                "bg": np.ascontiguousarray(
                    np.concatenate(
                        [b1[e].reshape(KH, P).T, g_e.reshape(C // P, P).T], axis=1
                    )
                ),
            }
        )

    # ---- run the expert MLPs on the 8 cores ----
    nc = _get_program(C, not b1.any())
    res = bass_utils.run_bass_kernel_spmd(nc, in_maps, core_ids=list(range(E)))
    ycat = np.concatenate([r["y"] for r in res.results], axis=0)  # [E*C, D]

    # ---- unshard: combine the two expert contributions per token ----
    comb = ycat[pos1] + ycat[pos2]  # gates already applied on device
    z = comb + tokens
    if b2.any():
        z += b2[idx1] * g1[:, None] + b2[idx2] * g2[:, None]

    # ---- residual + layernorm (host) ----
    mu = z.mean(axis=1, keepdims=True, dtype=np.float32)
    zc = z - mu
    var = np.mean(zc * zc, axis=1, keepdims=True, dtype=np.float32)
    out = zc * (1.0 / np.sqrt(var + LN_EPS)) * ln_gamma + ln_beta
    out = out.reshape(B, T, D).astype(np.float32)

    # ---- aux losses (host, exact) ----
    m = logits.max(axis=1, keepdims=True)
    ee = np.exp(logits - m)
    route_probs = ee / ee.sum(axis=1, keepdims=True)  # [N, E] f32
    importance = route_probs.mean(axis=0).astype(np.float32)  # [E]
    load = (counts / N).astype(np.float32)  # exact: N = 2**13
    balance_loss = np.float32(E * np.sum(importance * load))
    eps = np.float32(1e-8)
    entropy = np.float32(-(route_probs * np.log(route_probs + eps)).sum(axis=-1).mean())
    utilization_entropy = np.float32(-(load * np.log(load + eps)).sum())

    return out, balance_loss, entropy, utilization_entropy, load, importance


# revision 20
# speedup vs baseline: 1.2143x; 1.0000x over previous
"""MoE layer (top-2 of 8 experts) on 8 Trainium2 NeuronCores.

Strategy: expert-parallel. The router (an [N,8] matmul), top-k dispatch and
final unshard/combine run on host as the shard/unshard steps; each core runs
the expert MLP  g * gelu(X_e @ W1_e + b1_e) @ W2_e  for the tokens routed to
its expert (capacity-padded), at 1/4 the dense-equivalent FLOPs.

Device dataflow (per core):
  - activations kept transposed (feature dim on partitions) so both matmuls
    chain without transposes:
      mm1: W1 tiles stationary, X^T moving  -> H^T (h on partitions)
      gelu(+b1) fused on ScalarE,  PSUM -> SBUF
      mm2: H^T subtiles stationary, W2 moving -> Y token-major
      gate scale fused into the PSUM->SBUF copy on ScalarE
  - matmuls in float32r (full-rate fp32 streaming mode)
  - all device inputs are host-pre-arranged into SBUF-native layouts so each
    DMA is one long contiguous run per partition (fast descriptor gen)
  - w1 streams in 8 chunks with the first two token tiles' mm1 interleaved
    across chunks, so the PE tracks the weight DMA at full duty from ~4us
Host then combines the two expert contributions per token (+b2, +residual)
and applies the LayerNorm; aux router losses are exact host-side math.
"""

import numpy as np

B, T, D, H, E, TOPK = 4, 2048, 1024, 2048, 8, 2
N = B * T
LN_EPS = 1e-5
P = 128
NTOK = 384  # tokens per mm1 moving tile
MM_DTYPE = "float32r"  # "float32r" (fast, ~1e-4 rel) or "float32" (exact)

KD = D // P  # 8   k-tiles over D
KH = H // P  # 16  k-tiles over H
NHC = 16  # w1 DMA chunks (1 h-tile each)
HCH = KH // NHC  # h-tiles per chunk = 1
ND512 = D // 512  # 2

_CACHE = {}


def _build(C, b1_zero):
    import concourse.tile as tile
    from concourse import bacc, bass, mybir

    f32 = mybir.dt.float32
    fmm = getattr(mybir.dt, MM_DTYPE)
    NT = C // NTOK
    Gelu = mybir.ActivationFunctionType.Gelu
    Copy = mybir.ActivationFunctionType.Copy

    nc = bacc.Bacc("TRN2", target_bir_lowering=False, debug=False, num_devices=8)
    # all pre-arranged on host into SBUF-native layouts (partition dim = P)
    xt_d = nc.dram_tensor("xt", (NT, P, KD * NTOK), fmm, kind="ExternalInput").ap()
    w1_d = nc.dram_tensor("w1", (NHC, P, KD * HCH * P), fmm, kind="ExternalInput").ap()
    w2_d = nc.dram_tensor("w2", (ND512, P, KH * 512), fmm, kind="ExternalInput").ap()
    bg_d = nc.dram_tensor("bg", (P, KH + C // P), f32, kind="ExternalInput").ap()
    y_d = nc.dram_tensor("y", (C, D), f32, kind="ExternalOutput").ap()

    with tile.TileContext(nc) as tc:
        with (
            tc.tile_pool(name="wpool", bufs=1) as wpool,
            tc.tile_pool(name="xpool", bufs=2) as xpool,
            tc.tile_pool(name="hpool", bufs=2) as hpool,
            tc.tile_pool(name="ypool", bufs=1) as ypool,
            tc.tile_pool(name="ps1", bufs=4, space="PSUM") as ps1pool,
            tc.tile_pool(name="ps2", bufs=4, space="PSUM") as ps2pool,
        ):
            # weights stream on the sync engine; xts/y on the (idle) vector
            # engine so their issue+semaphore waves stay independent
            def dma_xts(tt):
                xts = xpool.tile([P, KD * NTOK], fmm, name="xts", tag="xts")
                nc.scalar.dma_start(xts[:], xt_d[tt])
                return xts

            # w1 chunk 0 first: the PE's first matmul needs only it + xts0
            w1cs = []

            def dma_w1c(hc):
                w1c = wpool.tile([P, KD * HCH * P], fmm, name=f"w1c{hc}")
                nc.sync.dma_start(w1c[:], w1_d[hc])
                w1cs.append(w1c)

            bgs = wpool.tile([P, KH + C // P], f32)
            nc.sync.dma_start(bgs[:], bg_d)
            b1s = bgs[:, :KH]
            gs = bgs[:, KH:]
            dma_w1c(0)
            NI = min(2, NT)
            xts_head = [dma_xts(t) for t in range(NI)]
            for hc in range(1, NHC):
                dma_w1c(hc)
            w2cs = []
            for dh in range(ND512):
                w2c = wpool.tile([P, KH * 512], fmm, name=f"w2c{dh}")
                nc.sync.dma_start(w2c[:], w2_d[dh])
                w2cs.append(w2c)

            # dummy matmuls on zeroed SBUF: keep the PE busy (and the HAM
            # clock-gate warm) while the first real input DMAs stream in
            dw = wpool.tile([P, P], f32)
            nc.gpsimd.memset(dw[:], 0.0)
            dx = wpool.tile([P, NTOK], f32)
            nc.gpsimd.memset(dx[:], 0.0)
            for _g in range(2):
                dps = ps1pool.tile([P, NTOK], f32, name="ps1s", tag="ps1s")
                for k in range(5):
                    nc.tensor.matmul(
                        dps[:], dw[:], dx[:], start=(k == 0), stop=(k == 4)
                    )

            hts_all = [None] * NT

            def mm1_hpair(hp, xts, hts):
                # two h-tiles accumulated into one PSUM bank, one gelu over
                # both (possible because b1 is zero -> shared scalar bias)
                ps1 = ps1pool.tile([P, 2 * NTOK], f32, name="ps1t", tag="ps1t")
                for half in range(2):
                    ht = 2 * hp + half
                    w1c = w1cs[ht // HCH]
                    hofs = (ht % HCH) * P
                    for kd in range(KD):
                        nc.tensor.matmul(
                            ps1[:, bass.ds(half * NTOK, NTOK)],
                            w1c[:, bass.ds(kd * HCH * P + hofs, P)],
                            xts[:, bass.ds(kd * NTOK, NTOK)],
                            start=(kd == 0),
                            stop=(kd == KD - 1),
                        )
                nc.scalar.activation(
                    hts[:, 2 * hp : 2 * hp + 2, :].rearrange("p a b -> p (a b)"),
                    ps1[:],
                    Gelu,
                )

            def mm1_htile(ht, xts, hts):
                ps1 = ps1pool.tile([P, NTOK], f32, name="ps1s", tag="ps1s")
                w1c = w1cs[ht // HCH]
                hofs = (ht % HCH) * P
                for kd in range(KD):
                    nc.tensor.matmul(
                        ps1[:],
                        w1c[:, bass.ds(kd * HCH * P + hofs, P)],
                        xts[:, bass.ds(kd * NTOK, NTOK)],
                        start=(kd == 0),
                        stop=(kd == KD - 1),
                    )
                nc.scalar.activation(
                    hts[:, ht, :], ps1[:], Gelu, bias=b1s[:, ht : ht + 1]
                )

            def mm1_chunk(hc, xts, hts):
                for ht in range(hc * HCH, (hc + 1) * HCH):
                    mm1_htile(ht, xts, hts)

            def emit_mm1(tt, xts):
                hts = hpool.tile([P, KH, NTOK], fmm, name="hts", tag="hts")
                for hc in range(NHC):
                    mm1_chunk(hc, xts, hts)
                hts_all[tt] = hts

            def emit_mm2(tt):
                hts = hts_all[tt]
                for sub in range(NTOK // P):
                    tsub = tt * (NTOK // P) + sub
                    ys = ypool.tile([P, D], f32, name="yst", tag="yst")
                    for dh in range(ND512):
                        ps2 = ps2pool.tile([P, 512], f32, name="ps2t", tag="ps2t")
                        for kh in range(KH):
                            nc.tensor.matmul(
                                ps2[:],
                                hts[:, kh, bass.ts(sub, P)],
                                w2cs[dh][:, bass.ds(kh * 512, 512)],
                                start=(kh == 0),
                                stop=(kh == KH - 1),
                            )
                        nc.vector.tensor_scalar_mul(
                            ys[:, bass.ts(dh, 512)],
                            ps2[:],
                            gs[:, tsub : tsub + 1],
                        )
                    nc.sync.dma_start(y_d[bass.ts(tsub, P), :], ys[:])
                hts_all[tt] = None

            # startup: interleave the first NI tiles' mm1 across w1 chunks so
            # the PE tracks the w1 DMA stream instead of waiting for all of it
            hts_head = [
                hpool.tile([P, KH, NTOK], fmm, name="hts", tag="hts")
                for _ in range(NI)
            ]
            for hc in range(NHC):
                for t in range(NI):
                    mm1_chunk(hc, xts_head[t], hts_head[t])
            for t in range(NI):
                hts_all[t] = hts_head[t]

            # steady-state software pipeline, NI tiles of mm1 ahead of mm2
            for tt in range(NI, NT):
                emit_mm2(tt - NI)
                xts = dma_xts(tt)
                emit_mm1(tt, xts)
            for tt in range(max(NT - NI, 0), NT):
                emit_mm2(tt)
    nc.compile()
    return nc


def _get_program(C, b1_zero):
    key = (C, b1_zero)
    if key not in _CACHE:
        _CACHE[key] = _build(C, b1_zero)
    return _CACHE[key]


def kernel(x, gate_W, gate_b, W1, b1, W2, b2, ln_gamma, ln_beta):
    from concourse import bass_utils

    x = np.asarray(x, dtype=np.float32)
    gate_W = np.asarray(gate_W, dtype=np.float32)
    gate_b = np.asarray(gate_b, dtype=np.float32)
    W1 = np.asarray(W1, dtype=np.float32)
    b1 = np.asarray(b1, dtype=np.float32)
    W2 = np.asarray(W2, dtype=np.float32)
    b2 = np.asarray(b2, dtype=np.float32)
    ln_gamma = np.asarray(ln_gamma, dtype=np.float32)
    ln_beta = np.asarray(ln_beta, dtype=np.float32)

    tokens = x.reshape(N, D)

    # ---- router (host; this is the dispatch/sharding step) ----
    logits = tokens @ gate_W + gate_b  # [N, E] f32
    ar = np.arange(N)
    idx1 = np.argmax(logits, axis=1)
    masked = logits.copy()
    masked[ar, idx1] = -np.inf
    idx2 = np.argmax(masked, axis=1)
    v1 = logits[ar, idx1]
    v2 = masked[ar, idx2]
    # softmax over the two top values (stable; v1 >= v2)
    e2 = np.exp((v2 - v1).astype(np.float32))
    g1 = (1.0 / (1.0 + e2)).astype(np.float32)
    g2 = (e2 / (1.0 + e2)).astype(np.float32)

    # ---- per-expert dispatch (capacity-padded so shapes are static) ----
    sel1 = [np.flatnonzero(idx1 == e) for e in range(E)]
    sel2 = [np.flatnonzero(idx2 == e) for e in range(E)]
    counts = np.array([len(a) + len(b) for a, b in zip(sel1, sel2)])
    C = max(int(np.max(counts)), 1)
    C = ((C + NTOK - 1) // NTOK) * NTOK
    NT = C // NTOK

    pos1 = np.empty(N, dtype=np.int64)
    pos2 = np.empty(N, dtype=np.int64)
    in_maps = []
    for e in range(E):
        a, bsel = sel1[e], sel2[e]
        idx_e = np.concatenate([a, bsel])
        g_e = np.zeros(C, dtype=np.float32)
        g_e[: len(a)] = g1[a]
        g_e[len(a) : len(idx_e)] = g2[bsel]
        pos1[a] = e * C + np.arange(len(a))
        pos2[bsel] = e * C + len(a) + np.arange(len(bsel))
        xg = np.zeros((C, D), dtype=np.float32)
        xg[: len(idx_e)] = tokens[idx_e]
        # [NT, P, KD*NTOK]: xt[tt, p, kd*NTOK+c] = xg[tt*NTOK+c, kd*P+p]
        xt = np.ascontiguousarray(
            xg.reshape(NT, NTOK, KD, P).transpose(0, 3, 2, 1)
        ).reshape(NT, P, KD * NTOK)
        # [NHC, P, KD*HCH*P]: w1[hc, p, kd*HCH*P+hh] = W1[e][kd*P+p, hc*HCH*P+hh]
        w1 = np.ascontiguousarray(
            W1[e].reshape(KD, P, NHC, HCH * P).transpose(2, 1, 0, 3)
        ).reshape(NHC, P, KD * HCH * P)
        # [ND512, P, KH*512]: w2[dh, p, kh*512+dd] = W2[e][kh*P+p, dh*512+dd]
        w2 = np.ascontiguousarray(
            W2[e].reshape(KH, P, ND512, 512).transpose(2, 1, 0, 3)
        ).reshape(ND512, P, KH * 512)
        in_maps.append(
            {
                "xt": xt,
                "w1": w1,
                "w2": w2,
                "bg": np.ascontiguousarray(
                    np.concatenate(
                        [b1[e].reshape(KH, P).T, g_e.reshape(C // P, P).T], axis=1
                    )
                ),
            }
        )

    # ---- run the expert MLPs on the 8 cores ----
    nc = _get_program(C, not b1.any())
    res = bass_utils.run_bass_kernel_spmd(nc, in_maps, core_ids=list(range(E)))
    ycat = np.concatenate([r["y"] for r in res.results], axis=0)  # [E*C, D]

    # ---- unshard: combine the two expert contributions per token ----
    comb = ycat[pos1] + ycat[pos2]  # gates already applied on device
    z = comb + tokens
    if b2.any():
        z += b2[idx1] * g1[:, None] + b2[idx2] * g2[:, None]

    # ---- residual + layernorm (host) ----
    mu = z.mean(axis=1, keepdims=True, dtype=np.float32)
    zc = z - mu
    var = np.mean(zc * zc, axis=1, keepdims=True, dtype=np.float32)
    out = zc * (1.0 / np.sqrt(var + LN_EPS)) * ln_gamma + ln_beta
    out = out.reshape(B, T, D).astype(np.float32)

    # ---- aux losses (host, exact) ----
    m = logits.max(axis=1, keepdims=True)
    ee = np.exp(logits - m)
    route_probs = ee / ee.sum(axis=1, keepdims=True)  # [N, E] f32
    importance = route_probs.mean(axis=0).astype(np.float32)  # [E]
    load = (counts / N).astype(np.float32)  # exact: N = 2**13
    balance_loss = np.float32(E * np.sum(importance * load))
    eps = np.float32(1e-8)
    entropy = np.float32(-(route_probs * np.log(route_probs + eps)).sum(axis=-1).mean())
    utilization_entropy = np.float32(-(load * np.log(load + eps)).sum())

    return out, balance_loss, entropy, utilization_entropy, load, importance
